# revision 3
# baseline (speedup 1.0000x reference)
"""
CollabFFLayer kernel for 8 TRN2 NeuronCores.

  y = relu(l2_normalize(x) @ W.T + b)     x:[8192,4096] W:[4096,4096] b:[4096]

Sharding: data-parallel over the batch dim. Core i processes rows
[1024*i, 1024*(i+1)); W and b are replicated. No collectives needed; the
host concatenates the 8 output shards.

Per-core dataflow (all compute in bf16 on the TensorEngine, fp32 accumulate):
  - x tiles [128,4096] are loaded, row-normalized (DVE sumsq -> sqrt ->
    +eps -> reciprocal -> ACT scale+cast to bf16), then XBAR
    dma-transposed into a resident x_nT laid out [i(128 part), k, b].
  - W is streamed: gpsimd (SWDGE) DMA casts fp32->bf16 on the way into
    SBUF in natural [o, i] layout, then XBAR SBUF->SBUF transposes
    produce W^T tiles [i(128 part), o(512)].
  - The bias is injected as a rank-1 K=1 matmul (ones[1,128].T @ b[1,512])
    that starts each PSUM accumulation group; 32 K=128 matmuls accumulate
    on top; the epilogue is a single ACT Relu PSUM->SBUF, then DMA out.
"""

import os

import numpy as np

B, IN, OUT = 8192, 4096, 4096
NCORES = 8
MB = B // NCORES  # 1024 batch rows per core
P = 128
KT = IN // P  # 32 contraction tiles
MT = MB // P  # 8 batch tiles per core
NF = 512  # matmul moving free dim (one PSUM bank of fp32)
NW = OUT // NF  # 8 output-feature windows
KK = 4  # k-tiles loaded per W DMA (1 MiB fp32 per transfer)

_CACHE = {}


def _build_nc(MB=MB, IN=IN, OUT=OUT):
    import concourse.mybir as mybir
    from concourse import bacc, tile

    KT = IN // P
    MT = MB // P
    NW = OUT // NF

    f32 = mybir.dt.float32
    bf16 = mybir.dt.bfloat16

    nc = bacc.Bacc("TRN2", target_bir_lowering=False, debug=False)

    x_d = nc.dram_tensor("x", [MB, IN], f32, kind="ExternalInput")
    w_d = nc.dram_tensor("W", [OUT, IN], f32, kind="ExternalInput")
    b_d = nc.dram_tensor("b", [1, OUT], f32, kind="ExternalInput")
    o_d = nc.dram_tensor("out", [MB, OUT], f32, kind="ExternalOutput")

    with tile.TileContext(nc) as tc:
        with (
            tc.tile_pool(name="const", bufs=1) as const,
            tc.tile_pool(name="xnt", bufs=1) as xnt_pool,
            tc.tile_pool(name="xin", bufs=2) as xin_pool,
            tc.tile_pool(name="xn", bufs=2) as xn_pool,
            tc.tile_pool(name="stats", bufs=2) as stats,
            tc.tile_pool(name="wraw", bufs=3) as wraw_pool,
            tc.tile_pool(name="wt", bufs=3) as wt_pool,
            tc.tile_pool(name="osb", bufs=4) as osb_pool,
            tc.tile_pool(name="psum", bufs=8, space="PSUM") as psum_pool,
        ):
            # ---- constants ----
            ones_sb = const.tile([1, P], bf16)
            nc.any.memset(ones_sb[:], 1.0)
            bias_sb = const.tile([1, OUT], bf16)
            nc.gpsimd.dma_start(out=bias_sb[:], in_=b_d[:])  # fp32 -> bf16 cast
            zero_bias = const.tile([P, 1], f32)
            nc.any.memset(zero_bias[:], 0.0)
            junk = const.tile([P, IN], bf16)  # ttr elementwise output (unused)

            # ---- x: load, l2-normalize, cast, transpose ----
            # x_nT[m][:, k, :] is the [i(128), b(128)] lhsT tile for (k, m)
            x_nT = [xnt_pool.tile([P, KT, P], bf16, name=f"x_nT_{m}") for m in range(MT)]
            for m in range(MT):
                xin = xin_pool.tile([P, IN], f32, tag="xin")
                nc.sync.dma_start(out=xin[:], in_=x_d[m * P : (m + 1) * P, :])
                ss = stats.tile([P, 1], f32, tag="ss")
                # NOTE: tensor_tensor_reduce wedges the device on this HW
                # path — scalar_tensor_tensor with accum_out computes the
                # same row sum-of-squares and runs fine.
                nc.vector.scalar_tensor_tensor(
                    out=junk[:],
                    in0=xin[:],
                    scalar=1.0,
                    in1=xin[:],
                    op0=mybir.AluOpType.mult,
                    op1=mybir.AluOpType.mult,
                    accum_out=ss[:],
                )
                nrm = stats.tile([P, 1], f32, tag="nrm")
                nc.scalar.sqrt(nrm[:], ss[:])
                nc.vector.tensor_scalar_add(nrm[:], nrm[:], 1e-8)
                rcp = stats.tile([P, 1], f32, tag="rcp")
                nc.vector.reciprocal(rcp[:], nrm[:])
                xn = xn_pool.tile([P, IN], bf16, tag="xn")
                nc.scalar.mul(xn[:], xin[:], rcp[:])
                for k in range(KT):
                    nc.sync.dma_start_transpose(
                        x_nT[m][:, k, :], xn[:, k * P : (k + 1) * P]
                    )

            # ---- main loop: stream W, matmul, epilogue ----
            for n in range(NW):
                psums = [
                    psum_pool.tile([P, NF], f32, tag="acc", name=f"ps_{n}_{m}")
                    for m in range(MT)
                ]
                # bias as rank-1 K=1 matmul starting the accumulation group
                for m in range(MT):
                    nc.tensor.matmul(
                        psums[m][:],
                        lhsT=ones_sb[:],
                        rhs=bias_sb[:, n * NF : (n + 1) * NF],
                        start=True,
                        stop=False,
                    )
                for ko in range(KT // KK):
                    # one SWDGE casting DMA: W[512 o, 512 i] fp32 -> bf16
                    # laid out [o_inner(128p), o_sub(4), k_sub(4), i(128)]
                    wraw = wraw_pool.tile([P, 4, KK, P], bf16, tag="wraw")
                    src = w_d[
                        n * NF : (n + 1) * NF, ko * KK * P : (ko + 1) * KK * P
                    ].rearrange("(s p) (kk i) -> p s kk i", p=P, i=P)
                    nc.gpsimd.dma_start(out=wraw[:], in_=src)
                    wt = wt_pool.tile([P, KK, NF], bf16, tag="wt")
                    for kk in range(KK):
                        for s in range(4):
                            nc.sync.dma_start_transpose(
                                wt[:, kk, s * P : (s + 1) * P], wraw[:, s, kk, :]
                            )
                    for kk in range(KK):
                        k = ko * KK + kk
                        last = k == KT - 1
                        for m in range(MT):
                            nc.tensor.matmul(
                                psums[m][:],
                                lhsT=x_nT[m][:, k, :],
                                rhs=wt[:, kk, :],
                                start=False,
                                stop=last,
                            )
                for m in range(MT):
                    osb = osb_pool.tile([P, NF], f32, tag="osb")
                    nc.scalar.activation(
                        osb[:],
                        psums[m][:],
                        mybir.ActivationFunctionType.Relu,
                        bias=zero_bias[:],
                    )
                    nc.sync.dma_start(
                        out=o_d[m * P : (m + 1) * P, n * NF : (n + 1) * NF],
                        in_=osb[:],
                    )

    nc.compile()
    return nc


def _get_nc():
    if "nc" not in _CACHE:
        os.environ.setdefault("MYCRO_LOCAL_CACHE", "1")
        _CACHE["nc"] = _build_nc()
    return _CACHE["nc"]


def kernel(x, W, b):
    from concourse.bass_utils import run_bass_kernel_spmd

    x = np.ascontiguousarray(np.asarray(x, dtype=np.float32))
    W = np.ascontiguousarray(np.asarray(W, dtype=np.float32))
    b = np.ascontiguousarray(np.asarray(b, dtype=np.float32)).reshape(1, OUT)
    assert x.shape == (B, IN) and W.shape == (OUT, IN)

    nc = _get_nc()
    in_maps = [
        {"x": np.ascontiguousarray(x[i * MB : (i + 1) * MB]), "W": W, "b": b}
        for i in range(NCORES)
    ]
    res = run_bass_kernel_spmd(nc, in_maps, core_ids=list(range(NCORES)))
    return np.concatenate([r["out"] for r in res.results], axis=0)


# revision 21
# speedup vs baseline: 16.1289x; 16.1289x over previous
"""
CollabFFLayer kernel for 8 TRN2 NeuronCores.

  y = relu(l2_normalize(x) @ W.T + b)     x:[8192,4096] W:[4096,4096] b:[4096]

Sharding: data-parallel over the batch dim. Core i processes rows
[1024*i, 1024*(i+1)); W and b are replicated. No collectives needed; the
host concatenates the 8 output shards.

Per-core dataflow (all compute in bf16 on the TensorEngine, fp32 accumulate):
  - x tiles [128,4096] are loaded, row-normalized (DVE sumsq -> sqrt ->
    +eps -> reciprocal -> ACT scale+cast to bf16), then XBAR
    dma-transposed into a resident x_nT laid out [i(128 part), k, b].
  - W is streamed: gpsimd (SWDGE) DMA casts fp32->bf16 on the way into
    SBUF in natural [o, i] layout, then XBAR SBUF->SBUF transposes
    produce W^T tiles [i(128 part), o(512)].
  - The bias is injected as a rank-1 K=1 matmul (ones[1,128].T @ b[1,512])
    that starts each PSUM accumulation group; 32 K=128 matmuls accumulate
    on top; the epilogue is a single ACT Relu PSUM->SBUF, then DMA out.
"""

import os

import numpy as np

B, IN, OUT = 8192, 4096, 4096
NCORES = 8
MB = B // NCORES  # 1024 batch rows per core
P = 128
NF = 512  # matmul moving free dim (one PSUM bank of fp32)
KK = 16  # k-tiles (128 each) loaded per W DMA (4 MiB fp32 per transfer)

_CACHE = {}


def _build_nc(MB=MB, IN=IN, OUT=OUT, reps=1, split_queues=False, hwdge_wcast=False,
              KKo=KK, wt_bufs=3, osb_bufs=6, out_gpsimd=False, wraw_bufs=2):
    import concourse.mybir as mybir
    from concourse import bacc, tile

    KT = IN // P
    KK = KKo
    MT = MB // P
    NW = OUT // NF

    f32 = mybir.dt.float32
    bf16 = mybir.dt.bfloat16

    nc = bacc.Bacc("TRN2", target_bir_lowering=False, debug=False)
    # transposes on the ACT HWDGE queue, plain DMAs on SP, to avoid
    # xbar-mode transitions serializing one queue
    tq = nc.scalar if split_queues else nc.sync

    x_d = nc.dram_tensor("x", [MB, IN], f32, kind="ExternalInput")
    w_d = nc.dram_tensor("W", [OUT, IN], f32, kind="ExternalInput")
    b_d = nc.dram_tensor("b", [1, OUT], f32, kind="ExternalInput")
    o_d = nc.dram_tensor("out", [MB, OUT], f32, kind="ExternalOutput")

    with tile.TileContext(nc) as tc:
        with (
            tc.tile_pool(name="const", bufs=1) as const,
            tc.tile_pool(name="xnt", bufs=1) as xnt_pool,
            tc.tile_pool(name="xin", bufs=2) as xin_pool,
            tc.tile_pool(name="xn", bufs=2) as xn_pool,
            tc.tile_pool(name="stats", bufs=2) as stats,
            tc.tile_pool(name="wraw", bufs=wraw_bufs) as wraw_pool,
            tc.tile_pool(name="wt", bufs=wt_bufs) as wt_pool,
            tc.tile_pool(name="osb", bufs=osb_bufs) as osb_pool,
            tc.tile_pool(name="psum", bufs=8, space="PSUM") as psum_pool,
            tc.tile_pool(name="dram", bufs=1, space="DRAM") as dram_pool,
        ):
            # ---- constants ----
            bias_sb = const.tile([1, OUT], bf16)
            nc.gpsimd.dma_start(out=bias_sb[:], in_=b_d[:])  # fp32 -> bf16 cast
            zero_bias = const.tile([P, 1], f32)
            nc.any.memset(zero_bias[:], 0.0)

            # reps>1 repeats the whole computation inside one NEFF, purely
            # for wall-clock device-time measurement:
            # t(reps=R) - t(reps=1) ~= (R-1) * t_kernel.
            for _rep in range(reps):
                # ---- x: load, l2-normalize, cast, transpose ----
                # x_nT[m][:, k, :] is the [i(128), b(128)] lhsT tile of (k, m)
                x_nT = [
                    xnt_pool.tile([P, KT, P], bf16, name=f"x_nT_{m}")
                    for m in range(MT)
                ]
                # Normalization is applied in the EPILOGUE (psum * rcp_b), so
                # the matmul runs on raw bf16 x and the prologue critical path
                # is just cast-DMA + transpose. The rank-1 bias matmul uses
                # lhsT = (norm+eps) so the epilogue scale cancels on the bias.
                rcps = [
                    stats.tile([P, 1], f32, name=f"rcp_{m}", bufs=1)
                    for m in range(MT)
                ]
                nrm_row = const.tile([1, MT * P], bf16, name=f"nrm_row_{_rep}")
                nrm_d = dram_pool.tile([MT, P], bf16, name=f"nrm_d_{_rep}")
                for m in range(MT):
                    # SWDGE casting DMA: x fp32 -> bf16 on load (halves x DMA
                    # bytes; bf16 rounding of x is far below the rel_err gate)
                    x16 = xin_pool.tile([P, IN], bf16, tag="x16")
                    nc.gpsimd.dma_start(out=x16[:], in_=x_d[m * P : (m + 1) * P, :])
                    # one XBAR instruction: [128 b, IN i] -> [128 i_in, KT, 128 b]
                    tq.dma_start_transpose(x_nT[m][:], x16[:])
                    ss = stats.tile([P, 1], f32, tag="ss")
                    scr = stats.tile([P, IN], bf16, tag="sumsq_scratch", bufs=1)
                    # NOTE: tensor_tensor_reduce wedges the device on this HW
                    # path — scalar_tensor_tensor computes the same row
                    # sum-of-squares and runs fine.
                    nc.vector.scalar_tensor_tensor(
                        out=scr[:],
                        in0=x16[:],
                        scalar=1.0,
                        in1=x16[:],
                        op0=mybir.AluOpType.mult,
                        op1=mybir.AluOpType.mult,
                        accum_out=ss[:],
                    )
                    nrm = stats.tile([P, 1], f32, tag="nrm")
                    nc.scalar.sqrt(nrm[:], ss[:])
                    nc.vector.tensor_scalar_add(nrm[:], nrm[:], 1e-8)
                    nc.vector.reciprocal(rcps[m][:], nrm[:])
                    nrm16 = stats.tile([P, 1], bf16, tag="nrm16")
                    nc.vector.tensor_copy(nrm16[:], nrm[:])
                    # partition->free move via a tiny DRAM bounce (a direct
                    # SBUF->SBUF transposing AP trips the race checker)
                    nc.sync.dma_start(out=nrm_d[m : m + 1, :], in_=nrm16[:])

                nc.sync.dma_start(
                    out=nrm_row[:], in_=nrm_d[:].rearrange("m p -> (m p)")[None, :]
                )

                # ---- main loop: stream W, matmul, epilogue ----
                for n in range(NW):
                    psums = [
                        psum_pool.tile([P, NF], f32, tag="acc", name=f"ps_{n}_{m}")
                        for m in range(MT)
                    ]
                    for ko in range(KT // KK):
                        # one SWDGE casting DMA: W[512 o, KK*128 i] fp32 -> bf16
                        # laid out [o_inner(128p), o_sub(4), k_sub(KK), i(128)];
                        # DRAM rows are read in KK*512B contiguous runs.
                        wraw = wraw_pool.tile([P, 4, KK, P], bf16, tag="wraw")
                        src = w_d[
                            n * NF : (n + 1) * NF, ko * KK * P : (ko + 1) * KK * P
                        ].rearrange("(s p) (kk i) -> p s kk i", p=P, i=P)
                        if hwdge_wcast:
                            wf = wraw_pool.tile([P, 4, KK, P], f32, tag="wf32")
                            nc.sync.dma_start(out=wf[:], in_=src)
                            nc.scalar.copy(wraw[:], wf[:])
                        else:
                            nc.gpsimd.dma_start(out=wraw[:], in_=src)
                        # 4 XBAR instructions: [128 o, KK*128 i] -> wt[:, :, s, :]
                        wt = wt_pool.tile([P, KK, 4, P], bf16, tag="wt")
                        for s in range(4):
                            tq.dma_start_transpose(
                                wt[:, :, s, :], wraw[:, s, :, :]
                            )
                        for kk in range(KK):
                            k = ko * KK + kk
                            for m in range(MT):
                                nc.tensor.matmul(
                                    psums[m][:],
                                    lhsT=x_nT[m][:, k, :],
                                    rhs=wt[:, kk, :, :],
                                    start=(k == 0),
                                    stop=False,
                                )
                    # bias as rank-1 K=1 matmul closing each accumulation
                    # group: psum += (norm_b + eps) * bias_o
                    for m in range(MT):
                        nc.tensor.matmul(
                            psums[m][:],
                            lhsT=nrm_row[:, m * P : (m + 1) * P],
                            rhs=bias_sb[:, n * NF : (n + 1) * NF],
                            start=False,
                            stop=True,
                        )
                    for m in range(MT):
                        osb = osb_pool.tile([P, NF], f32, tag="osb")
                        # alternate engines so the PSUM drain at window
                        # boundaries is 2-wide
                        if m % 2 == 0:
                            nc.scalar.activation(
                                osb[:],
                                psums[m][:],
                                mybir.ActivationFunctionType.Relu,
                                bias=zero_bias[:],
                                scale=rcps[m][:],
                            )
                        else:
                            nc.vector.tensor_scalar(
                                out=osb[:],
                                in0=psums[m][:],
                                scalar1=rcps[m][:],
                                scalar2=0.0,
                                op0=mybir.AluOpType.mult,
                                op1=mybir.AluOpType.max,
                            )
                        (nc.gpsimd if out_gpsimd else nc.sync).dma_start(
                            out=o_d[m * P : (m + 1) * P, n * NF : (n + 1) * NF],
                            in_=osb[:],
                        )

    nc.compile()
    return nc


def _get_nc():
    if "nc" not in _CACHE:
        os.environ.setdefault("MYCRO_LOCAL_CACHE", "1")
        _CACHE["nc"] = _build_nc()
    return _CACHE["nc"]


def kernel(x, W, b):
    from concourse.bass_utils import run_bass_kernel_spmd

    x = np.ascontiguousarray(np.asarray(x, dtype=np.float32))
    W = np.ascontiguousarray(np.asarray(W, dtype=np.float32))
    b = np.ascontiguousarray(np.asarray(b, dtype=np.float32)).reshape(1, OUT)
    assert x.shape == (B, IN) and W.shape == (OUT, IN)

    nc = _get_nc()
    in_maps = [
        {"x": np.ascontiguousarray(x[i * MB : (i + 1) * MB]), "W": W, "b": b}
        for i in range(NCORES)
    ]
    res = run_bass_kernel_spmd(nc, in_maps, core_ids=list(range(NCORES)))
    return np.concatenate([r["out"] for r in res.results], axis=0)
